# revision 1
# baseline (speedup 1.0000x reference)
"""CrossEntropyLoss kernel for Trainium2, SPMD over 8 NeuronCores.

reference:
    gathered = output[i, label[i]]                      # [B]
    loss = (sum_i -gathered_i + sum_i log(sum_j exp(output[i,j]) + 1e-5)) / B

Sharding: batch (B=8192) split across 8 cores, 1024 rows per core.
Per core: stream the [1024, 32000] f32 shard from HBM in [128, 8000]
chunks; ACT engine computes exp with fused row-sum accumulation
(accum_out); the label gather uses one indirect DMA over flattened
indices; ln(sumexp + eps) - gathered per row goes back to the host,
which sums and divides by B.
"""

import numpy as np

import concourse.bass as bass
import concourse.mybir as mybir
import concourse.tile as tile
from concourse.bass_utils import run_bass_kernel_spmd

B, V = 8192, 32000
N_CORES = 8
B_LOC = B // N_CORES  # 1024 rows per core
P = 128  # SBUF partitions
EPS = 1e-5


def split_multi_waits(nc):
    """This walrus build's CoreV2/V3 codegen rejects any instruction carrying
    more than one sync wait command. Split extra waits onto same-engine NoOps
    inserted immediately before the offending instruction (sequential waits on
    one engine are equivalent to one AND-ed wait set)."""
    n_split = 0
    for func in nc.m.functions:
        for block in func.blocks:
            new_insts = []
            for inst in block.instructions:
                si = inst.sync_info
                if si is not None and len(si.on_wait) > 1:
                    waits = list(si.on_wait)
                    for w in waits[:-1]:
                        nop = mybir.InstNoOp(
                            name=f"I-waitsplit-{nc.next_id()}",
                            sync_info=mybir.SyncInfo(on_wait=[w], on_update=[]),
                            bass_nofuse=True,
                            engine=inst.engine,
                        )
                        nc.register_instruction(nop)
                        new_insts.append(nop)
                        n_split += 1
                    si.on_wait = [waits[-1]]
                new_insts.append(inst)
            block.instructions[:] = new_insts
    return n_split


def build_nc(b_loc=B_LOC, v=V, dma_chunk=8000, act_chunk=4000, xin_bufs=3, repeat=1):
    """Build the single-core Bass program (same program runs SPMD on all cores).

    repeat>1 re-runs the streaming phase (identical work/results) so one
    dispatch holds R x the device work - used only for timing measurements.
    """
    assert b_loc % P == 0 and v % dma_chunk == 0 and dma_chunk % act_chunk == 0
    n_rt = b_loc // P  # row tiles of 128 rows
    n_dc = v // dma_chunk  # DMA chunks per row tile
    spc = dma_chunk // act_chunk  # ACT sub-chunks per DMA chunk
    n_ch = n_rt * n_dc * spc  # total ACT chunks

    nc = bass.Bass()
    x = nc.dram_tensor("x", [b_loc, v], mybir.dt.float32, kind="ExternalInput")
    idx = nc.dram_tensor("idx", [P, n_rt], mybir.dt.int32, kind="ExternalInput")
    out = nc.dram_tensor("out", [P, n_rt], mybir.dt.float32, kind="ExternalOutput")

    x_flat = x[:].rearrange("a (b one) -> (a b) one", one=1)

    with tile.TileContext(nc) as tc:
        with (
            tc.tile_pool(name="xin", bufs=xin_bufs) as xin,
            tc.tile_pool(name="trash", bufs=1, space="PSUM") as trash,
            tc.tile_pool(name="small", bufs=1) as small,
        ):
            # Label gather: overlaps with the streaming loop (reads DRAM only).
            idx_t = small.tile([P, n_rt], mybir.dt.int32)
            nc.sync.dma_start(out=idx_t[:], in_=idx[:])
            g_t = small.tile([P, n_rt], mybir.dt.float32)
            # One [128,1] gather per row tile: multi-column offset APs
            # mis-address on HW (verified), per-column gathers are exact.
            for rt in range(n_rt):
                nc.gpsimd.indirect_dma_start(
                    out=g_t[:, rt : rt + 1],
                    out_offset=None,
                    in_=x_flat,
                    in_offset=bass.IndirectOffsetOnAxis(
                        ap=idx_t[:, rt : rt + 1], axis=0
                    ),
                )

            # partials[p, rt*n_dc*spc + c] = sum over one act_chunk of exp(x)
            partials = small.tile([P, n_ch], mybir.dt.float32)
            for _rep in range(repeat):
              for rt in range(n_rt):
                for dc in range(n_dc):
                    x_t = xin.tile([P, dma_chunk], mybir.dt.float32, tag="x")
                    nc.sync.dma_start(
                        out=x_t[:],
                        in_=x[rt * P : (rt + 1) * P, dc * dma_chunk : (dc + 1) * dma_chunk],
                    )
                    for s in range(spc):
                        e_t = trash.tile([P, act_chunk], mybir.dt.float32, tag="e")
                        c = (rt * n_dc + dc) * spc + s
                        nc.scalar.activation(
                            out=e_t[:],
                            in_=x_t[:, s * act_chunk : (s + 1) * act_chunk],
                            func=mybir.ActivationFunctionType.Exp,
                            accum_out=partials[:, c : c + 1],
                        )

            # Combine: sumexp per row -> ln(. + eps) -> minus gathered logit.
            sums = small.tile([P, n_rt], mybir.dt.float32)
            cpr = n_dc * spc  # chunks per row tile
            for rt in range(n_rt):
                nc.vector.reduce_sum(
                    out=sums[:, rt : rt + 1],
                    in_=partials[:, rt * cpr : (rt + 1) * cpr],
                    axis=mybir.AxisListType.X,
                )
            eps_t = small.tile([P, 1], mybir.dt.float32)
            nc.gpsimd.memset(eps_t[:], EPS)
            lg_t = small.tile([P, n_rt], mybir.dt.float32)
            nc.scalar.activation(
                out=lg_t[:],
                in_=sums[:],
                func=mybir.ActivationFunctionType.Ln,
                bias=eps_t[:],
            )
            res_t = small.tile([P, n_rt], mybir.dt.float32)
            nc.vector.tensor_sub(out=res_t[:], in0=lg_t[:], in1=g_t[:])
            nc.sync.dma_start(out=out[:], in_=res_t[:])

    split_multi_waits(nc)
    return nc


def make_in_maps(output, label, b_loc=B_LOC, v=V, n_cores=N_CORES):
    """Shard full inputs into per-core input maps."""
    output = np.asarray(output)
    label = np.asarray(label).astype(np.int64)
    n_rt = b_loc // P
    in_maps = []
    for c in range(n_cores):
        xs = np.ascontiguousarray(output[c * b_loc : (c + 1) * b_loc], dtype=np.float32)
        ls = label[c * b_loc : (c + 1) * b_loc]
        flat = (np.arange(b_loc, dtype=np.int64) * v + ls).astype(np.int32)
        idx_mat = np.ascontiguousarray(flat.reshape(n_rt, P).T)  # [p, rt]
        in_maps.append({"x": xs, "idx": idx_mat})
    return in_maps


def combine(results, b=B):
    """Sum per-row terms from all cores and divide by the batch size."""
    total = 0.0
    for r in results:
        total += r["out"].astype(np.float64).sum()
    return np.float32(total / b)


_NC_CACHE = {}


def kernel(output, label):
    if "nc" not in _NC_CACHE:
        _NC_CACHE["nc"] = build_nc()
    nc = _NC_CACHE["nc"]
    in_maps = make_in_maps(output, label)
    res = run_bass_kernel_spmd(nc, in_maps, list(range(N_CORES)))
    return combine(res.results)



# revision 2
# speedup vs baseline: 2.1091x; 2.1091x over previous
"""CrossEntropyLoss kernel for Trainium2, SPMD over 8 NeuronCores.

reference:
    gathered = output[i, label[i]]                      # [B]
    loss = (sum_i -gathered_i + sum_i log(sum_j exp(output[i,j]) + 1e-5)) / B

Sharding: batch (B=8192) split across 8 cores, 1024 rows per core.

Per core: stream the [1024, 32000] f32 shard from HBM in [128, 8000]
chunks (4 MB HWDGE transfers, triple-buffered). Each chunk's columns are
split between two engines so exp+row-sum compute never gates the DMA
stream:
  - ACT (scalar engine) takes C_ACT columns: exp with fused row-sum
    accumulation (accum_out), junk elementwise output to PSUM.
  - DVE (vector engine) takes C_DVE columns via a Schraudolph-style
    fast exp: tensor_scalar computes i16 = int16(x*(2^7*log2 e) + B16)
    (runs at 2x: fp32 single-src), whose int16 bit pattern reinterpreted
    as bfloat16 is a ~1% accurate exp(x) (the linear-interp exponent
    trick); a second tensor_scalar (bypass, 4x: bf16) reduces the
    bitcast view with accum_out. B16 is calibrated so the weighted mean
    relative error of the row-sum is ~0 (loss error ~5e-4 relative,
    tolerance 2e-2).
The label gather uses one indirect DMA per 128-row tile over flattened
indices; ln(sumexp + eps) - gathered per row goes back to the host,
which sums and divides by B.
"""

import numpy as np

import concourse.bass as bass
import concourse.mybir as mybir
import concourse.tile as tile
from concourse.bass_utils import run_bass_kernel_spmd

B, V = 8192, 32000
N_CORES = 8
B_LOC = B // N_CORES  # 1024 rows per core
P = 128  # SBUF partitions
EPS = 1e-5

# Schraudolph bf16 constants: i16 = int16(x * A16 + B16); bitcast to bf16.
A16 = 184.66496  # 2^7 * log2(e)
B16 = 16249.0  # 127*2^7 - C, C calibrated for zero mean log-sum-exp error


def split_multi_waits(nc):
    """This walrus build's CoreV2/V3 codegen rejects any instruction carrying
    more than one sync wait command. Split extra waits onto same-engine NoOps
    inserted immediately before the offending instruction (sequential waits on
    one engine are equivalent to one AND-ed wait set)."""
    n_split = 0
    for func in nc.m.functions:
        for block in func.blocks:
            new_insts = []
            for inst in block.instructions:
                si = inst.sync_info
                if si is not None and len(si.on_wait) > 1:
                    waits = list(si.on_wait)
                    for w in waits[:-1]:
                        nop = mybir.InstNoOp(
                            name=f"I-waitsplit-{nc.next_id()}",
                            sync_info=mybir.SyncInfo(on_wait=[w], on_update=[]),
                            bass_nofuse=True,
                            engine=inst.engine,
                        )
                        nc.register_instruction(nop)
                        new_insts.append(nop)
                        n_split += 1
                    si.on_wait = [waits[-1]]
                new_insts.append(inst)
            block.instructions[:] = new_insts
    return n_split


def build_nc(
    b_loc=B_LOC,
    v=V,
    dma_chunk=8000,
    c_dve=4160,
    xin_bufs=3,
    repeat=1,
):
    """Build the single-core Bass program (same program runs SPMD on all cores).

    repeat>1 re-runs the streaming phase (identical work/results) so one
    dispatch holds R x the device work - used only for timing measurements.
    """
    assert b_loc % P == 0 and v % dma_chunk == 0
    c_act = dma_chunk - c_dve
    n_rt = b_loc // P  # row tiles of 128 rows
    n_dc = v // dma_chunk  # DMA chunks per row tile
    n_ch = n_rt * n_dc  # total chunks (one ACT + one DVE pipeline each)

    nc = bass.Bass()
    x = nc.dram_tensor("x", [b_loc, v], mybir.dt.float32, kind="ExternalInput")
    idx = nc.dram_tensor("idx", [P, n_rt], mybir.dt.int32, kind="ExternalInput")
    out = nc.dram_tensor("out", [P, n_rt], mybir.dt.float32, kind="ExternalOutput")

    x_flat = x[:].rearrange("a (b one) -> (a b) one", one=1)

    with tile.TileContext(nc) as tc:
        with (
            tc.tile_pool(name="xin", bufs=xin_bufs) as xin,
            tc.tile_pool(name="i16p", bufs=2) as i16p,
            tc.tile_pool(name="trash", bufs=1, space="PSUM") as trash,
            tc.tile_pool(name="junk", bufs=1) as junkp,
            tc.tile_pool(name="small", bufs=1) as small,
        ):
            # Label gather: overlaps with the streaming loop (reads DRAM only).
            idx_t = small.tile([P, n_rt], mybir.dt.int32)
            nc.sync.dma_start(out=idx_t[:], in_=idx[:])
            g_t = small.tile([P, n_rt], mybir.dt.float32)
            # One [128,1] gather per row tile: multi-column offset APs
            # mis-address on HW (verified), per-column gathers are exact.
            for rt in range(n_rt):
                nc.gpsimd.indirect_dma_start(
                    out=g_t[:, rt : rt + 1],
                    out_offset=None,
                    in_=x_flat,
                    in_offset=bass.IndirectOffsetOnAxis(
                        ap=idx_t[:, rt : rt + 1], axis=0
                    ),
                )

            # Per-chunk partial sums of exp(x): ACT part and DVE part.
            pa = small.tile([P, n_ch], mybir.dt.float32)
            pd = small.tile([P, n_ch], mybir.dt.float32)
            junk16 = junkp.tile([P, c_dve], mybir.dt.bfloat16)
            for _rep in range(repeat):
                for rt in range(n_rt):
                    for dc in range(n_dc):
                        c = rt * n_dc + dc
                        x_t = xin.tile([P, dma_chunk], mybir.dt.float32, tag="x")
                        nc.sync.dma_start(
                            out=x_t[:],
                            in_=x[
                                rt * P : (rt + 1) * P,
                                dc * dma_chunk : (dc + 1) * dma_chunk,
                            ],
                        )
                        # ACT: exp over the first c_act columns, fused row-sum.
                        e_t = trash.tile([P, c_act], mybir.dt.float32, tag="e")
                        nc.scalar.activation(
                            out=e_t[:],
                            in_=x_t[:, 0:c_act],
                            func=mybir.ActivationFunctionType.Exp,
                            accum_out=pa[:, c : c + 1],
                        )
                        # DVE: fast-exp bit trick over the remaining columns.
                        i16 = i16p.tile([P, c_dve], mybir.dt.int16, tag="i")
                        nc.vector.tensor_scalar(
                            out=i16[:],
                            in0=x_t[:, c_act:dma_chunk],
                            scalar1=A16,
                            scalar2=B16,
                            op0=mybir.AluOpType.mult,
                            op1=mybir.AluOpType.add,
                        )
                        nc.vector.tensor_scalar(
                            out=junk16[:],
                            in0=i16[:].bitcast(mybir.dt.bfloat16),
                            scalar1=0.0,
                            scalar2=None,
                            op0=mybir.AluOpType.bypass,
                            op1=mybir.AluOpType.add,
                            accum_out=pd[:, c : c + 1],
                        )

            # Combine: sumexp per row -> ln(. + eps) -> minus gathered logit.
            ps = small.tile([P, n_ch], mybir.dt.float32)
            nc.vector.tensor_add(out=ps[:], in0=pa[:], in1=pd[:])
            sums = small.tile([P, n_rt], mybir.dt.float32)
            for rt in range(n_rt):
                nc.vector.reduce_sum(
                    out=sums[:, rt : rt + 1],
                    in_=ps[:, rt * n_dc : (rt + 1) * n_dc],
                    axis=mybir.AxisListType.X,
                )
            eps_t = small.tile([P, 1], mybir.dt.float32)
            nc.gpsimd.memset(eps_t[:], EPS)
            lg_t = small.tile([P, n_rt], mybir.dt.float32)
            nc.scalar.activation(
                out=lg_t[:],
                in_=sums[:],
                func=mybir.ActivationFunctionType.Ln,
                bias=eps_t[:],
            )
            res_t = small.tile([P, n_rt], mybir.dt.float32)
            nc.vector.tensor_sub(out=res_t[:], in0=lg_t[:], in1=g_t[:])
            nc.sync.dma_start(out=out[:], in_=res_t[:])

    split_multi_waits(nc)
    return nc


def make_in_maps(output, label, b_loc=B_LOC, v=V, n_cores=N_CORES):
    """Shard full inputs into per-core input maps."""
    output = np.asarray(output)
    label = np.asarray(label).astype(np.int64)
    n_rt = b_loc // P
    in_maps = []
    for c in range(n_cores):
        xs = np.ascontiguousarray(output[c * b_loc : (c + 1) * b_loc], dtype=np.float32)
        ls = label[c * b_loc : (c + 1) * b_loc]
        flat = (np.arange(b_loc, dtype=np.int64) * v + ls).astype(np.int32)
        idx_mat = np.ascontiguousarray(flat.reshape(n_rt, P).T)  # [p, rt]
        in_maps.append({"x": xs, "idx": idx_mat})
    return in_maps


def combine(results, b=B):
    """Sum per-row terms from all cores and divide by the batch size."""
    total = 0.0
    for r in results:
        total += r["out"].astype(np.float64).sum()
    return np.float32(total / b)


_NC_CACHE = {}


def kernel(output, label):
    if "nc" not in _NC_CACHE:
        _NC_CACHE["nc"] = build_nc()
    nc = _NC_CACHE["nc"]
    in_maps = make_in_maps(output, label)
    res = run_bass_kernel_spmd(nc, in_maps, list(range(N_CORES)))
    return combine(res.results)


# revision 6
# speedup vs baseline: 2.7833x; 1.3197x over previous
"""CrossEntropyLoss kernel for Trainium2, SPMD over 8 NeuronCores.

reference:
    gathered = output[i, label[i]]                      # [B]
    loss = (sum_i -gathered_i + sum_i log(sum_j exp(output[i,j]) + 1e-5)) / B

Sharding: batch (B=8192) split across 8 cores, 1024 rows per core.

Per core: stream the [1024, 32000] f32 shard from HBM. Two variants:

  bf16 path (default): the DMA casts f32 -> bf16 in flight (SWDGE), so
  the SBUF-fabric side moves half the bytes; the HBM read side is
  unchanged.  This lifts the single-core DMA ceiling from the ~435 GB/s
  SBUF-AXI port limit toward the HBM-stack limit.

  Each chunk's columns are split between two engines so exp+row-sum
  compute never gates the DMA stream:
  - ACT (scalar engine): exp with fused row-sum accumulation
    (accum_out), junk elementwise output to PSUM.
  - DVE (vector engine): Schraudolph-style fast exp in the bf16 bit
    domain: tensor_scalar computes i16 = int16(x*(2^7*log2 e) + B16),
    whose bit pattern reinterpreted as bfloat16 is a ~2%-max-error
    exp(x) (the linear-interp exponent trick); a second tensor_scalar
    (bypass) reduces the bitcast view via accum_out.  B16 is calibrated
    so the exp-weighted mean relative error is ~0; per-row averaging
    over 32000 samples leaves ~1e-4 noise on each log-sum-exp (loss
    tolerance is 2e-2).

The label gather uses one indirect DMA per 128-row tile over flattened
f32 indices (exact values); ln(sumexp + eps) - gathered per row goes
back to the host, which sums and divides by B.
"""

import numpy as np

import concourse.bass as bass
import concourse.mybir as mybir
import concourse.tile as tile
from concourse.bass_utils import run_bass_kernel_spmd

B, V = 8192, 32000
N_CORES = 8
B_LOC = B // N_CORES  # 1024 rows per core
P = 128  # SBUF partitions
EPS = 1e-5

# Schraudolph bf16 constants: i16 = int16(x * A16 + B16); bitcast to bf16.
A16 = 184.66496  # 2^7 * log2(e)
B16 = 16249.0  # 127*2^7 - C, C calibrated for zero mean log-sum-exp error


def split_multi_waits(nc):
    """This walrus build's CoreV2/V3 codegen rejects any instruction carrying
    more than one sync wait command. Split extra waits onto same-engine NoOps
    inserted immediately before the offending instruction (sequential waits on
    one engine are equivalent to one AND-ed wait set)."""
    n_split = 0
    for func in nc.m.functions:
        for block in func.blocks:
            new_insts = []
            for inst in block.instructions:
                si = inst.sync_info
                if si is not None and len(si.on_wait) > 1:
                    waits = list(si.on_wait)
                    for w in waits[:-1]:
                        nop = mybir.InstNoOp(
                            name=f"I-waitsplit-{nc.next_id()}",
                            sync_info=mybir.SyncInfo(on_wait=[w], on_update=[]),
                            bass_nofuse=True,
                            engine=inst.engine,
                        )
                        nc.register_instruction(nop)
                        new_insts.append(nop)
                        n_split += 1
                    si.on_wait = [waits[-1]]
                new_insts.append(inst)
            block.instructions[:] = new_insts
    return n_split


def build_nc(
    b_loc=B_LOC,
    v=V,
    dma_chunk=16000,
    c_dve=5120,
    xin_bufs=3,
    use_bf16=True,
    repeat=1,
):
    """Build the single-core Bass program (same program runs SPMD on all cores).

    repeat>1 re-runs the streaming phase (identical work/results) so one
    dispatch holds R x the device work - used only for timing measurements.
    """
    assert b_loc % P == 0 and v % dma_chunk == 0
    c_act = dma_chunk - c_dve
    assert c_act % 2 == 0 and c_dve % 2 == 0
    n_rt = b_loc // P  # row tiles of 128 rows
    n_dc = v // dma_chunk  # DMA chunks per row tile
    n_ch = n_rt * n_dc  # total chunks (one ACT + one DVE pipeline each)
    x_dt = mybir.dt.bfloat16 if use_bf16 else mybir.dt.float32

    nc = bass.Bass()
    x = nc.dram_tensor("x", [b_loc, v], mybir.dt.float32, kind="ExternalInput")
    idx = nc.dram_tensor("idx", [P, n_rt], mybir.dt.int32, kind="ExternalInput")
    out = nc.dram_tensor("out", [P, n_rt], mybir.dt.float32, kind="ExternalOutput")

    x_flat = x[:].rearrange("a (b one) -> (a b) one", one=1)

    with tile.TileContext(nc) as tc:
        with (
            tc.tile_pool(name="xin", bufs=xin_bufs) as xin,
            tc.tile_pool(name="i16p", bufs=2) as i16p,
            tc.tile_pool(name="junk", bufs=1) as junkp,
            tc.tile_pool(name="small", bufs=1) as small,
        ):
            # Label gather: overlaps with the streaming loop (reads DRAM only).
            idx_t = small.tile([P, n_rt], mybir.dt.int32)
            nc.sync.dma_start(out=idx_t[:], in_=idx[:])
            g_t = small.tile([P, n_rt], mybir.dt.float32)
            # One [128,1] gather per row tile: multi-column offset APs
            # mis-address on HW (verified), per-column gathers are exact.
            for rt in range(n_rt):
                nc.gpsimd.indirect_dma_start(
                    out=g_t[:, rt : rt + 1],
                    out_offset=None,
                    in_=x_flat,
                    in_offset=bass.IndirectOffsetOnAxis(
                        ap=idx_t[:, rt : rt + 1], axis=0
                    ),
                )

            # Per-chunk partial sums of exp(x): ACT part and DVE part.
            pa = small.tile([P, n_ch], mybir.dt.float32)
            pd = small.tile([P, n_ch], mybir.dt.float32)
            junk16 = junkp.tile([P, c_dve], mybir.dt.bfloat16)
            junk_act = junkp.tile([P, c_act], mybir.dt.bfloat16)
            for _rep in range(repeat):
                for rt in range(n_rt):
                    for dc in range(n_dc):
                        c = rt * n_dc + dc
                        x_t = xin.tile([P, dma_chunk], x_dt, tag="x")
                        src = x[
                            rt * P : (rt + 1) * P,
                            dc * dma_chunk : (dc + 1) * dma_chunk,
                        ]
                        if use_bf16:
                            nc.gpsimd.dma_start(out=x_t[:], in_=src)
                        else:
                            nc.sync.dma_start(out=x_t[:], in_=src)
                        # ACT: exp over the first c_act columns, fused row-sum.
                        nc.scalar.activation(
                            out=junk_act[:],
                            in_=x_t[:, 0:c_act],
                            func=mybir.ActivationFunctionType.Exp,
                            accum_out=pa[:, c : c + 1],
                        )
                        # DVE: fast-exp bit trick over the remaining columns.
                        i16 = i16p.tile([P, c_dve], mybir.dt.int16, tag="i")
                        nc.vector.tensor_scalar(
                            out=i16[:],
                            in0=x_t[:, c_act:dma_chunk],
                            scalar1=A16,
                            scalar2=B16,
                            op0=mybir.AluOpType.mult,
                            op1=mybir.AluOpType.add,
                        )
                        nc.vector.tensor_scalar(
                            out=junk16[:],
                            in0=i16[:].bitcast(mybir.dt.bfloat16),
                            scalar1=0.0,
                            scalar2=None,
                            op0=mybir.AluOpType.bypass,
                            op1=mybir.AluOpType.add,
                            accum_out=pd[:, c : c + 1],
                        )

            # Combine: sumexp per row -> ln(. + eps) -> minus gathered logit.
            ps = small.tile([P, n_ch], mybir.dt.float32)
            nc.vector.tensor_add(out=ps[:], in0=pa[:], in1=pd[:])
            sums = small.tile([P, n_rt], mybir.dt.float32)
            for rt in range(n_rt):
                nc.vector.reduce_sum(
                    out=sums[:, rt : rt + 1],
                    in_=ps[:, rt * n_dc : (rt + 1) * n_dc],
                    axis=mybir.AxisListType.X,
                )
            eps_t = small.tile([P, 1], mybir.dt.float32)
            nc.gpsimd.memset(eps_t[:], EPS)
            lg_t = small.tile([P, n_rt], mybir.dt.float32)
            nc.scalar.activation(
                out=lg_t[:],
                in_=sums[:],
                func=mybir.ActivationFunctionType.Ln,
                bias=eps_t[:],
            )
            res_t = small.tile([P, n_rt], mybir.dt.float32)
            nc.vector.tensor_sub(out=res_t[:], in0=lg_t[:], in1=g_t[:])
            nc.sync.dma_start(out=out[:], in_=res_t[:])

    split_multi_waits(nc)
    return nc


def make_in_maps(output, label, b_loc=B_LOC, v=V, n_cores=N_CORES):
    """Shard full inputs into per-core input maps."""
    output = np.asarray(output)
    label = np.asarray(label).astype(np.int64)
    n_rt = b_loc // P
    in_maps = []
    for c in range(n_cores):
        xs = np.ascontiguousarray(output[c * b_loc : (c + 1) * b_loc], dtype=np.float32)
        ls = label[c * b_loc : (c + 1) * b_loc]
        flat = (np.arange(b_loc, dtype=np.int64) * v + ls).astype(np.int32)
        idx_mat = np.ascontiguousarray(flat.reshape(n_rt, P).T)  # [p, rt]
        in_maps.append({"x": xs, "idx": idx_mat})
    return in_maps


def combine(results, b=B):
    """Sum per-row terms from all cores and divide by the batch size."""
    total = 0.0
    for r in results:
        total += r["out"].astype(np.float64).sum()
    return np.float32(total / b)


_NC_CACHE = {}


def kernel(output, label):
    if "nc" not in _NC_CACHE:
        _NC_CACHE["nc"] = build_nc()
    nc = _NC_CACHE["nc"]
    in_maps = make_in_maps(output, label)
    res = run_bass_kernel_spmd(nc, in_maps, list(range(N_CORES)))
    return combine(res.results)


# revision 8
# speedup vs baseline: 2.9315x; 1.0532x over previous
"""CrossEntropyLoss kernel for Trainium2, SPMD over 8 NeuronCores.

reference:
    gathered = output[i, label[i]]                      # [B]
    loss = (sum_i -gathered_i + sum_i log(sum_j exp(output[i,j]) + 1e-5)) / B

Sharding: batch (B=8192) split across 8 cores, 1024 rows per core.

Per core: stream the [1024, 32000] f32 shard from HBM. Two variants:

  bf16 path (default): the DMA casts f32 -> bf16 in flight (SWDGE), so
  the SBUF-fabric side moves half the bytes; the HBM read side is
  unchanged.  This lifts the single-core DMA ceiling from the ~435 GB/s
  SBUF-AXI port limit toward the HBM-stack limit.

  Each chunk's columns are split between two engines so exp+row-sum
  compute never gates the DMA stream:
  - ACT (scalar engine): exp with fused row-sum accumulation
    (accum_out), junk elementwise output to PSUM.
  - DVE (vector engine): Schraudolph-style fast exp in the bf16 bit
    domain: tensor_scalar computes i16 = int16(x*(2^7*log2 e) + B16),
    whose bit pattern reinterpreted as bfloat16 is a ~2%-max-error
    exp(x) (the linear-interp exponent trick); a second tensor_scalar
    (bypass) reduces the bitcast view via accum_out.  B16 is calibrated
    so the exp-weighted mean relative error is ~0; per-row averaging
    over 32000 samples leaves ~1e-4 noise on each log-sum-exp (loss
    tolerance is 2e-2).

The label gather uses one indirect DMA per 128-row tile over flattened
f32 indices (exact values); ln(sumexp + eps) - gathered per row goes
back to the host, which sums and divides by B.
"""

import numpy as np

import concourse.bass as bass
import concourse.mybir as mybir
import concourse.tile as tile
from concourse.bass_utils import run_bass_kernel_spmd

B, V = 8192, 32000
N_CORES = 8
B_LOC = B // N_CORES  # 1024 rows per core
P = 128  # SBUF partitions
EPS = 1e-5

# Schraudolph bf16 constants: i16 = int16(x * A16 + B16); bitcast to bf16.
A16 = 184.66496  # 2^7 * log2(e)
B16 = 16249.135  # 127*2^7 - C, C calibrated for zero mean log-sum-exp error


def split_multi_waits(nc):
    """This walrus build's CoreV2/V3 codegen rejects any instruction carrying
    more than one sync wait command. Split extra waits onto same-engine NoOps
    inserted immediately before the offending instruction (sequential waits on
    one engine are equivalent to one AND-ed wait set)."""
    n_split = 0
    for func in nc.m.functions:
        for block in func.blocks:
            new_insts = []
            for inst in block.instructions:
                si = inst.sync_info
                if si is not None and len(si.on_wait) > 1:
                    waits = list(si.on_wait)
                    for w in waits[:-1]:
                        nop = mybir.InstNoOp(
                            name=f"I-waitsplit-{nc.next_id()}",
                            sync_info=mybir.SyncInfo(on_wait=[w], on_update=[]),
                            bass_nofuse=True,
                            engine=inst.engine,
                        )
                        nc.register_instruction(nop)
                        new_insts.append(nop)
                        n_split += 1
                    si.on_wait = [waits[-1]]
                new_insts.append(inst)
            block.instructions[:] = new_insts
    return n_split


def build_nc(
    b_loc=B_LOC,
    v=V,
    dma_chunk=16000,
    c_dve=5888,
    xin_bufs=3,
    use_bf16=True,
    repeat=1,
):
    """Build the single-core Bass program (same program runs SPMD on all cores).

    repeat>1 re-runs the streaming phase (identical work/results) so one
    dispatch holds R x the device work - used only for timing measurements.
    """
    assert b_loc % P == 0 and v % dma_chunk == 0
    c_act = dma_chunk - c_dve
    assert c_act % 2 == 0 and c_dve % 2 == 0
    n_rt = b_loc // P  # row tiles of 128 rows
    n_dc = v // dma_chunk  # DMA chunks per row tile
    n_ch = n_rt * n_dc  # total chunks (one ACT + one DVE pipeline each)
    x_dt = mybir.dt.bfloat16 if use_bf16 else mybir.dt.float32

    nc = bass.Bass()
    x = nc.dram_tensor("x", [b_loc, v], mybir.dt.float32, kind="ExternalInput")
    idx = nc.dram_tensor("idx", [P, n_rt], mybir.dt.int32, kind="ExternalInput")
    out = nc.dram_tensor("out", [P, n_rt], mybir.dt.float32, kind="ExternalOutput")

    x_flat = x[:].rearrange("a (b one) -> (a b) one", one=1)

    with tile.TileContext(nc) as tc:
        with (
            tc.tile_pool(name="xin", bufs=xin_bufs) as xin,
            tc.tile_pool(name="i16p", bufs=2) as i16p,
            tc.tile_pool(name="junk", bufs=1) as junkp,
            tc.tile_pool(name="small", bufs=1) as small,
        ):
            # Label gather: overlaps with the streaming loop (reads DRAM only).
            idx_t = small.tile([P, n_rt], mybir.dt.int32)
            nc.sync.dma_start(out=idx_t[:], in_=idx[:])
            g_t = small.tile([P, n_rt], mybir.dt.float32)
            # One [128,1] gather per row tile: multi-column offset APs
            # mis-address on HW (verified), per-column gathers are exact.
            for rt in range(n_rt):
                nc.gpsimd.indirect_dma_start(
                    out=g_t[:, rt : rt + 1],
                    out_offset=None,
                    in_=x_flat,
                    in_offset=bass.IndirectOffsetOnAxis(
                        ap=idx_t[:, rt : rt + 1], axis=0
                    ),
                )

            # Per-chunk partial sums of exp(x): ACT part and DVE part.
            pa = small.tile([P, n_ch], mybir.dt.float32)
            pd = small.tile([P, n_ch], mybir.dt.float32)
            junk16 = junkp.tile([P, c_dve], mybir.dt.bfloat16)
            junk_act = junkp.tile([P, c_act], mybir.dt.bfloat16)
            for _rep in range(repeat):
                for rt in range(n_rt):
                    for dc in range(n_dc):
                        c = rt * n_dc + dc
                        x_t = xin.tile([P, dma_chunk], x_dt, tag="x")
                        src = x[
                            rt * P : (rt + 1) * P,
                            dc * dma_chunk : (dc + 1) * dma_chunk,
                        ]
                        if use_bf16:
                            nc.gpsimd.dma_start(out=x_t[:], in_=src)
                        else:
                            nc.sync.dma_start(out=x_t[:], in_=src)
                        # ACT: exp over the first c_act columns, fused row-sum.
                        nc.scalar.activation(
                            out=junk_act[:],
                            in_=x_t[:, 0:c_act],
                            func=mybir.ActivationFunctionType.Exp,
                            accum_out=pa[:, c : c + 1],
                        )
                        # DVE: fast-exp bit trick over the remaining columns.
                        i16 = i16p.tile([P, c_dve], mybir.dt.int16, tag="i")
                        nc.vector.tensor_scalar(
                            out=i16[:],
                            in0=x_t[:, c_act:dma_chunk],
                            scalar1=A16,
                            scalar2=B16,
                            op0=mybir.AluOpType.mult,
                            op1=mybir.AluOpType.add,
                        )
                        nc.vector.tensor_scalar(
                            out=junk16[:],
                            in0=i16[:].bitcast(mybir.dt.bfloat16),
                            scalar1=0.0,
                            scalar2=None,
                            op0=mybir.AluOpType.bypass,
                            op1=mybir.AluOpType.add,
                            accum_out=pd[:, c : c + 1],
                        )

            # Combine: sumexp per row -> ln(. + eps) -> minus gathered logit.
            ps = small.tile([P, n_ch], mybir.dt.float32)
            nc.vector.tensor_add(out=ps[:], in0=pa[:], in1=pd[:])
            sums = small.tile([P, n_rt], mybir.dt.float32)
            for rt in range(n_rt):
                nc.vector.reduce_sum(
                    out=sums[:, rt : rt + 1],
                    in_=ps[:, rt * n_dc : (rt + 1) * n_dc],
                    axis=mybir.AxisListType.X,
                )
            eps_t = small.tile([P, 1], mybir.dt.float32)
            nc.gpsimd.memset(eps_t[:], EPS)
            lg_t = small.tile([P, n_rt], mybir.dt.float32)
            nc.scalar.activation(
                out=lg_t[:],
                in_=sums[:],
                func=mybir.ActivationFunctionType.Ln,
                bias=eps_t[:],
            )
            res_t = small.tile([P, n_rt], mybir.dt.float32)
            nc.vector.tensor_sub(out=res_t[:], in0=lg_t[:], in1=g_t[:])
            nc.sync.dma_start(out=out[:], in_=res_t[:])

    split_multi_waits(nc)
    return nc


def make_in_maps(output, label, b_loc=B_LOC, v=V, n_cores=N_CORES):
    """Shard full inputs into per-core input maps."""
    output = np.asarray(output)
    label = np.asarray(label).astype(np.int64)
    n_rt = b_loc // P
    in_maps = []
    for c in range(n_cores):
        xs = np.ascontiguousarray(output[c * b_loc : (c + 1) * b_loc], dtype=np.float32)
        ls = label[c * b_loc : (c + 1) * b_loc]
        flat = (np.arange(b_loc, dtype=np.int64) * v + ls).astype(np.int32)
        idx_mat = np.ascontiguousarray(flat.reshape(n_rt, P).T)  # [p, rt]
        in_maps.append({"x": xs, "idx": idx_mat})
    return in_maps


def combine(results, b=B):
    """Sum per-row terms from all cores and divide by the batch size."""
    total = 0.0
    for r in results:
        total += r["out"].astype(np.float64).sum()
    return np.float32(total / b)


_NC_CACHE = {}


def kernel(output, label):
    if "nc" not in _NC_CACHE:
        _NC_CACHE["nc"] = build_nc()
    nc = _NC_CACHE["nc"]
    in_maps = make_in_maps(output, label)
    res = run_bass_kernel_spmd(nc, in_maps, list(range(N_CORES)))
    return combine(res.results)
